# revision 42
# baseline (speedup 1.0000x reference)
"""ALNN layer kernel for 8 TRN2 NeuronCores (raw Bass, explicit semaphores).

out[b,r,d] = relu( sum_l w_v[r,l,d]*relu(z[b,r,l,d]) + L*b_v[r,d] )
z = wt0*X + wt1*relu(X)*k + wt2*M + wt3*PD + 4*bt
k = exp(-relu(alpha_r)*|T - s_r|)        (uses relu(X*k) == relu(X)*k, k>0)

Sharding: B split 2 ways x R split 4 ways -> 8 cores, each 16 b x 12 r.
Per-core layout: partitions = L(128), free = (b=16, d=64) = 1024.

v2 engine plan (v1 profiling: GpSimd shares an SBUF port with DVE — an
exclusive lock — so streaming on POOL poisoned DVE ops 677ns -> 2600ns):
 - DVE: only the 5 elementwise products (+ wl = w_v*lat), bf16 2x mode
 - PE:  z = bt4 + q + m0 + m2 + m3 via identity-matmul PSUM accumulation
        (bt4 host-expanded to [L, B*D] as the PSUM initializer), plus the
        final L-reduction as one-hot-column matmuls into PSUM row r
 - ACT: dist=abs, k=exp, lat=relu(PSUM z) (ACT is closest to PSUM)
 - POOL: nothing.

Raw bass: this toolchain's walrus allows at most ONE attached sync-wait
per compute instruction, so cross-engine deps use standalone wait_ge
instructions; DMA completion uses per-stream/per-slot semaphores (HW DMA
queues complete out of order, so one shared counting sem is unsound).
"""

import os
import numpy as np
import ml_dtypes

import concourse.bass as bass
import concourse.mybir as mybir
from concourse.bass_utils import run_bass_kernel_spmd

AF = mybir.ActivationFunctionType
OP = mybir.AluOpType
BF16 = mybir.dt.bfloat16
F32 = mybir.dt.float32

B, R, L, D = 32, 48, 128, 64
NB, NR = 2, 4              # b-blocks x r-blocks = 8 cores
BC, RC = B // NB, R // NR  # 16, 12 per core
FD = BC * D                # 1024 free elems

# packed f32 const layout: [Tt 1024 | Abc 12 | NASbc 12 | LBV 64(rows 0..11)]
CF_W = FD + RC + RC + D
# packed bf16 const layout: [Xt | Mt | PDt | OH 144 | I 128]
CB_W = 3 * FD + RC * RC + L
# per-r param slice: [wt0|wt1|wt2|wt3|wv] 5*64 + bt4 expanded to [L, FD]
WS_W = 5 * D + FD

_nbf16 = ml_dtypes.bfloat16

KB = 3   # k buffers (ACT -> DVE)
WB = 5   # ws slots (DMA -> DVE/PE)
LB = 5   # wl buffers (DVE -> PE)
LAB = 4  # lat buffers (ACT -> DVE)
PRB = 3  # product buffer sets (DVE -> PE)
ZB = 3   # psum z triple-buffer (uses 6 of 8 banks; ps0/ps1 take the rest)
WLAG = 3  # wl(r-WLAG) emitted in DVE iter r
ALAG = 2  # lat(r-ALAG) emitted in ACT iter r


def _vc_m3(r):
    if r < WLAG:
        return 5 * r + 6
    return 6 * r + 4


def _vc_wl(rr):
    if rr <= RC - 1 - WLAG:
        return 6 * (rr + WLAG) - 1
    return 73 - (RC - 1 - rr)


def _vc_g(r):
    if r < WLAG:
        return 5 * r + 2
    return 6 * r


def _build_graph():
    nc = bass.Bass()
    cf_e = nc.declare_dram_parameter("cf", [L, CF_W], F32, isOutput=False)
    cb_e = nc.declare_dram_parameter("cb", [L, CB_W], BF16, isOutput=False)
    Wp_e = nc.declare_dram_parameter("Wp", [RC, L, WS_W], BF16, isOutput=False)
    out_e = nc.declare_dram_parameter("out", [RC, FD], F32, isOutput=True)

    from contextlib import ExitStack

    with ExitStack() as ctx:
        e = ctx.enter_context
        cf = e(nc.sbuf_tensor([L, CF_W], F32))
        cb = e(nc.sbuf_tensor([L, CB_W], BF16))
        Xp = e(nc.sbuf_tensor([L, FD], BF16))
        dist = e(nc.sbuf_tensor([L, FD], F32))
        kbuf = e(nc.sbuf_tensor([L, KB * FD], BF16))
        wsbuf = e(nc.sbuf_tensor([L, WB * WS_W], BF16))
        g = e(nc.sbuf_tensor([L, FD], BF16))
        qb = e(nc.sbuf_tensor([L, PRB * FD], BF16))
        m0b = e(nc.sbuf_tensor([L, PRB * FD], BF16))
        m2b = e(nc.sbuf_tensor([L, PRB * FD], BF16))
        m3b = e(nc.sbuf_tensor([L, PRB * FD], BF16))
        latb = e(nc.sbuf_tensor([L, LAB * FD], BF16))
        wlbuf = e(nc.sbuf_tensor([L, LB * FD], BF16))
        ob = e(nc.sbuf_tensor([RC, FD], F32))
        outt = e(nc.sbuf_tensor([RC, FD], F32))
        psz = [e(nc.psum_tensor(f"psz{j}", [L, FD], F32)) for j in range(ZB)]
        ps0 = e(nc.psum_tensor([RC, 512], F32))
        ps1 = e(nc.psum_tensor([RC, 512], F32))
        cfsem = e(nc.semaphore("cfsem"))
        cbsem = e(nc.semaphore("cbsem"))
        wsem = [e(nc.semaphore(f"wsem{j}")) for j in range(WB)]
        asem = e(nc.semaphore("asem"))   # ACT k completions (1/r)
        lsem = e(nc.semaphore("lsem"))   # ACT lat completions (1/r)
        zsem = e(nc.semaphore("zsem"))   # PE z-group completions (1/r)
        msem = e(nc.semaphore("msem"))   # PE out-mm completions (1/r)
        vsem = e(nc.semaphore("vsem"))   # DVE op completions
        osem = e(nc.semaphore("osem"))
        block = e(nc.Block())

        Tt = cf[:, 0:FD]
        Abc = cf[:, FD : FD + RC]
        NASbc = cf[:, FD + RC : FD + 2 * RC]
        LBVt = cf[0:RC, FD + 2 * RC : FD + 2 * RC + D]
        Xt = cb[:, 0:FD]
        Mt = cb[:, FD : 2 * FD]
        PDt = cb[:, 2 * FD : 3 * FD]
        OH3 = cb[:, 3 * FD : 3 * FD + RC * RC].rearrange("p (r m) -> p r m", r=RC)
        Ident = cb[:, 3 * FD + RC * RC :]

        def r3(ap):
            return ap.rearrange("p (b d) -> p b d", b=BC)

        def kslot(r):
            return kbuf[:, (r % KB) * FD : (r % KB + 1) * FD]

        def wslot(r):
            return wsbuf[:, (r % WB) * WS_W : (r % WB + 1) * WS_W]

        def wbc(r, e):
            base = (r % WB) * WS_W + e * D
            return wsbuf[:, base : base + D].unsqueeze(1).broadcast_to([L, BC, D])

        def bt4x(r):
            base = (r % WB) * WS_W + 5 * D
            return wsbuf[:, base : base + FD]

        def latslot(r):
            return latb[:, (r % LAB) * FD : (r % LAB + 1) * FD]

        def wlslot(r):
            return wlbuf[:, (r % LB) * FD : (r % LB + 1) * FD]

        def prod(buf, r):
            return buf[:, (r % PRB) * FD : (r % PRB + 1) * FD]

        CBH = CB_W // 2

        @block.sync
        def _(sp):
            for r in range(RC):
                if r == 1:
                    sp.dma_start(
                        out=cb[:, 0:CBH], in_=cb_e[:, 0:CBH]
                    ).then_inc(cbsem, 16)
                    sp.dma_start(
                        out=cb[:, CBH:], in_=cb_e[:, CBH:]
                    ).then_inc(cbsem, 16)
                if r >= WB:
                    # ws slot readers: DVE wl(r-WB) is the last DVE read;
                    # PE z-group(r-WB) reads bt4x
                    sp.wait_ge(vsem, _vc_wl(r - WB))
                    sp.wait_ge(zsem, r - WB + 1)
                sp.dma_start(out=wslot(r), in_=Wp_e[r, :, :]).then_inc(
                    wsem[r % WB], 16
                )
            sp.wait_ge(vsem, 76)
            sp.dma_start(out=out_e[:, :], in_=outt[:, :]).then_inc(osem, 16)

        @block.scalar
        def _(act):
            act.dma_start(out=cf[:, :], in_=cf_e[:, :]).then_inc(cfsem, 16)
            act.wait_ge(cfsem, 16)
            for r in range(RC):
                nc.scalar.activation(
                    dist[:, :], Tt, AF.Abs,
                    bias=NASbc[:, r : r + 1], scale=Abc[:, r : r + 1],
                )
                if r >= KB:
                    act.wait_ge(vsem, _vc_g(r - KB))
                nc.scalar.activation(
                    kslot(r), dist[:, :], AF.Exp, scale=-1.0
                ).then_inc(asem, 1)
                if r >= ALAG:
                    rr = r - ALAG
                    act.wait_ge(zsem, rr + 1)
                    if rr >= LAB:
                        act.wait_ge(vsem, _vc_wl(rr - LAB))
                    nc.scalar.activation(
                        latslot(rr), psz[rr % ZB][:, :], AF.Relu
                    ).then_inc(lsem, 1)
            for rr in range(RC - ALAG, RC):
                act.wait_ge(zsem, rr + 1)
                nc.scalar.activation(
                    latslot(rr), psz[rr % ZB][:, :], AF.Relu
                ).then_inc(lsem, 1)

        @block.vector
        def _(ve):
            ve.wait_ge(cbsem, 32)
            nc.vector.tensor_scalar_max(Xp[:, :], Xt, 0.0).then_inc(vsem, 1)
            for r in range(RC):
                if r >= WLAG:
                    rr = r - WLAG
                    # wl(rr): lat(rr) is WLAG iterations old; the lsem wait
                    # also implies zsem >= r-2, covering product-slot reuse
                    ve.wait_ge(lsem, rr + 1)
                    if rr >= LB:
                        ve.wait_ge(msem, rr - LB + 1)
                    nc.vector.tensor_tensor(
                        r3(wlslot(rr)), r3(latslot(rr)), wbc(rr, 4),
                        OP.mult,
                    ).then_inc(vsem, 1)
                ve.wait_ge(asem, r + 1)
                nc.vector.tensor_mul(g[:, :], Xp[:, :], kslot(r)).then_inc(vsem, 1)
                ve.wait_ge(wsem[r % WB], 16 * (r // WB + 1))
                nc.vector.tensor_tensor(
                    r3(prod(qb, r)), r3(g[:, :]), wbc(r, 1), OP.mult
                ).then_inc(vsem, 1)
                nc.vector.tensor_tensor(
                    r3(prod(m0b, r)), r3(Xt), wbc(r, 0), OP.mult
                ).then_inc(vsem, 1)
                nc.vector.tensor_tensor(
                    r3(prod(m2b, r)), r3(Mt), wbc(r, 2), OP.mult
                ).then_inc(vsem, 1)
                nc.vector.tensor_tensor(
                    r3(prod(m3b, r)), r3(PDt), wbc(r, 3), OP.mult
                ).then_inc(vsem, 1)
            # final wl's + tail
            for rr in range(RC - WLAG, RC):
                ve.wait_ge(lsem, rr + 1)
                ve.wait_ge(msem, rr - LB + 1)
                nc.vector.tensor_tensor(
                    r3(wlslot(rr)), r3(latslot(rr)), wbc(rr, 4), OP.mult
                ).then_inc(vsem, 1)
            ve.wait_ge(msem, RC)
            ve.wait_ge(cfsem, 16)
            lb3 = LBVt.unsqueeze(1).broadcast_to([RC, BC // 2, D])
            for h, ps in enumerate((ps0, ps1)):
                ob3 = r3(ob[:, :])[:, h * (BC // 2) : (h + 1) * (BC // 2), :]
                ps3 = ps[:, :].rearrange("p (b d) -> p b d", b=BC // 2)
                nc.vector.scalar_tensor_tensor(
                    ob3, ps3, 0.0, lb3, OP.add, OP.add
                ).then_inc(vsem, 1)
            nc.vector.tensor_scalar_max(outt[:, :], ob[:, :], 0.0).then_inc(vsem, 1)

        @block.tensor
        def _(te):
            te.wait_ge(cbsem, 32)
            # HAM warmup: ~12 throwaway matmuls on already-loaded SBUF so the
            # PE clock is at 2.4GHz when the first real z-group arrives; ps0
            # is clobbered but the real accumulation restarts with start=True
            for _w in range(12):
                nc.tensor.matmul(
                    ps0[:, :], OH3[:, 0, :], cb[:, 0:512],
                    start=True, stop=True, skip_group_check=True,
                )
            for r in range(RC):
                te.wait_ge(vsem, _vc_m3(r))
                if r >= ZB:
                    te.wait_ge(lsem, r - ZB + 1)
                pz = psz[r % ZB]
                # alternate PSUM banks between consecutive matmuls so the
                # drain of one overlaps the fill of the next
                for pb, first, last in (
                    (None, True, False),
                    (qb, False, False),
                    (m0b, False, False),
                    (m2b, False, False),
                    (m3b, False, True),
                ):
                    for h in range(2):
                        c0, c1 = h * 512, (h + 1) * 512
                        rhs = (
                            bt4x(r)[:, c0:c1]
                            if pb is None
                            else prod(pb, r)[:, c0:c1]
                        )
                        mm = nc.tensor.matmul(
                            pz[:, c0:c1], Ident, rhs,
                            start=first, stop=last, skip_group_check=True,
                        )
                        if last and h == 1:
                            mm.then_inc(zsem, 1)
                if r >= WLAG:
                    rr = r - WLAG
                    te.wait_ge(vsem, _vc_wl(rr))
                    wl = wlslot(rr)
                    nc.tensor.matmul(
                        ps0[:, :], OH3[:, rr, :], wl[:, 0:512],
                        start=(rr == 0), stop=False,
                        skip_group_check=True,
                    )
                    nc.tensor.matmul(
                        ps1[:, :], OH3[:, rr, :], wl[:, 512:1024],
                        start=(rr == 0), stop=False,
                        skip_group_check=True,
                    ).then_inc(msem, 1)
            for rr in range(RC - WLAG, RC):
                te.wait_ge(vsem, _vc_wl(rr))
                wl = wlslot(rr)
                nc.tensor.matmul(
                    ps0[:, :], OH3[:, rr, :], wl[:, 0:512],
                    start=False, stop=(rr == RC - 1), skip_group_check=True,
                )
                nc.tensor.matmul(
                    ps1[:, :], OH3[:, rr, :], wl[:, 512:1024],
                    start=False, stop=(rr == RC - 1), skip_group_check=True,
                ).then_inc(msem, 1)

    return nc


_CACHE = {}


def kernel(X, T, M, PD, alpha, w_v, w_t, b_t, b_v, ref_time):
    X = np.asarray(X, np.float32)
    T = np.asarray(T, np.float32)
    M = np.asarray(M, np.float32)
    PD = np.asarray(PD, np.float32)
    alpha = np.asarray(alpha, np.float32)
    w_v = np.asarray(w_v, np.float32)
    w_t = np.asarray(w_t, np.float32)
    b_t = np.asarray(b_t, np.float32)
    b_v = np.asarray(b_v, np.float32)
    ref_time = np.asarray(ref_time, np.float32)

    a = np.maximum(alpha.reshape(R), 0.0)
    s_ref = ref_time.reshape(R)
    nas = -(a * s_ref)
    bt4 = 4.0 * b_t[..., 0]              # [R, L, D]
    lbv = float(L) * b_v[:, 0, :]        # [R, D]

    # per-r params: [wt0|wt1|wt2|wt3|wv] (5*D) + bt4 expanded to [L, FD]
    wts = np.stack(
        [w_t[..., 0], w_t[..., 1], w_t[..., 2], w_t[..., 3], w_v], axis=2
    )                                     # [R, L, 5, D]
    bt4x = np.broadcast_to(bt4[:, :, None, :], (R, L, BC, D)).reshape(R, L, FD)
    wpack = np.concatenate(
        [wts.reshape(R, L, 5 * D), bt4x], axis=2
    )                                     # [R, L, WS_W]

    oh = np.zeros((L, RC, RC), np.float32)
    for r in range(RC):
        oh[:, r, r] = 1.0
    ident = np.eye(L, dtype=np.float32)

    if "nc" not in _CACHE:
        _CACHE["nc"] = _build_graph()
    nc = _CACHE["nc"]

    in_maps = []
    for c in range(8):
        b0 = (c // NR) * BC
        r0 = (c % NR) * RC
        tr = lambda x: np.ascontiguousarray(
            x[b0 : b0 + BC].transpose(1, 0, 2).reshape(L, FD)
        )
        cf = np.zeros((L, CF_W), np.float32)
        cf[:, 0:FD] = tr(T)
        cf[:, FD : FD + RC] = a[r0 : r0 + RC]
        cf[:, FD + RC : FD + 2 * RC] = nas[r0 : r0 + RC]
        cf[0:RC, FD + 2 * RC : FD + 2 * RC + D] = lbv[r0 : r0 + RC]
        cbf = np.zeros((L, CB_W), np.float32)
        cbf[:, 0:FD] = tr(X)
        cbf[:, FD : 2 * FD] = tr(M)
        cbf[:, 2 * FD : 3 * FD] = tr(PD)
        cbf[:, 3 * FD : 3 * FD + RC * RC] = oh.reshape(L, RC * RC)
        cbf[:, 3 * FD + RC * RC :] = ident
        in_maps.append(
            {
                "cf": cf,
                "cb": cbf.astype(_nbf16),
                "Wp": np.ascontiguousarray(wpack[r0 : r0 + RC]).astype(_nbf16),
            }
        )

    trace = bool(os.environ.get("BASS_KERNEL_TRACE"))
    kw = {}
    if trace:
        tmpdir = os.environ.get("BASS_KERNEL_TRACE_DIR") or None
        kw = dict(trace=True, tmpdir=tmpdir)
    res = run_bass_kernel_spmd(nc, in_maps, core_ids=list(range(8)), **kw)
    if trace:
        _CACHE["exec_time_ns"] = res.exec_time_ns
        print(f"HW exec time: {res.exec_time_ns} ns")

    out = np.zeros((B, R, D), np.float32)
    for c in range(8):
        b0 = (c // NR) * BC
        r0 = (c % NR) * RC
        o = np.asarray(res.results[c]["out"], np.float32).reshape(RC, BC, D)
        out[b0 : b0 + BC, r0 : r0 + RC] = o.transpose(1, 0, 2)
    return out


# revision 45
# speedup vs baseline: 1.1113x; 1.1113x over previous
"""ALNN layer kernel for 8 TRN2 NeuronCores (raw Bass, explicit semaphores).

out[b,r,d] = relu( sum_l w_v[r,l,d]*relu(z[b,r,l,d]) + L*b_v[r,d] )
z = wt0*X + wt1*relu(X)*k + wt2*M + wt3*PD + 4*bt
k = exp(-relu(alpha_r)*|T - s_r|)        (uses relu(X*k) == relu(X)*k, k>0)

Sharding: B split 2 ways x R split 4 ways -> 8 cores, each 16 b x 12 r.
Per-core layout: partitions = L(128), free = (b=16, d=64) = 1024.

v2 engine plan (v1 profiling: GpSimd shares an SBUF port with DVE — an
exclusive lock — so streaming on POOL poisoned DVE ops 677ns -> 2600ns):
 - DVE: only the 5 elementwise products (+ wl = w_v*lat), bf16 2x mode
 - PE:  z = bt4 + q + m0 + m2 + m3 via identity-matmul PSUM accumulation
        (bt4 host-expanded to [L, B*D] as the PSUM initializer), plus the
        final L-reduction as one-hot-column matmuls into PSUM row r
 - ACT: dist=abs, k=exp, lat=relu(PSUM z) (ACT is closest to PSUM)
 - POOL: nothing.

Raw bass: this toolchain's walrus allows at most ONE attached sync-wait
per compute instruction, so cross-engine deps use standalone wait_ge
instructions; DMA completion uses per-stream/per-slot semaphores (HW DMA
queues complete out of order, so one shared counting sem is unsound).
"""

import os
import numpy as np
import ml_dtypes

import concourse.bass as bass
import concourse.mybir as mybir
from concourse.bass_utils import run_bass_kernel_spmd

AF = mybir.ActivationFunctionType
OP = mybir.AluOpType
BF16 = mybir.dt.bfloat16
F32 = mybir.dt.float32

B, R, L, D = 32, 48, 128, 64
NB, NR = 2, 4              # b-blocks x r-blocks = 8 cores
BC, RC = B // NB, R // NR  # 16, 12 per core
FD = BC * D                # 1024 free elems

# packed f32 const layout: [Tt 1024 | Abc 12 | NASbc 12 | LBV 64(rows 0..11)]
CF_W = FD + RC + RC + D
# packed bf16 const layout: [Xt | Mt | PDt | OH 144 | I 128]
CB_W = 3 * FD + RC * RC + L
# per-r param slice: [wt0|wt1|wt2|wt3|wv] 5*64 + bt4 expanded to [L, FD]
WS_W = 5 * D + FD

_nbf16 = ml_dtypes.bfloat16

KB = 3   # k buffers (ACT -> DVE)
WB = 5   # ws slots (DMA -> DVE/PE)
LB = 5   # wl buffers (DVE -> PE)
LAB = 4  # lat buffers (ACT -> DVE)
PRB = 3  # product buffer sets (DVE -> PE)
ZB = 3   # psum z triple-buffer (uses 6 of 8 banks; ps0/ps1 take the rest)
WLAG = 3  # wl(r-WLAG) emitted in DVE iter r
ALAG = 2  # lat(r-ALAG) emitted in ACT iter r


def _vc_m3(r):
    if r < WLAG:
        return 5 * r + 6
    return 6 * r + 4


def _vc_wl(rr):
    if rr <= RC - 1 - WLAG:
        return 6 * (rr + WLAG) - 1
    return 73 - (RC - 1 - rr)


def _vc_g(r):
    if r < WLAG:
        return 5 * r + 2
    return 6 * r


def _build_graph():
    nc = bass.Bass()
    cf_e = nc.declare_dram_parameter("cf", [L, CF_W], F32, isOutput=False)
    cb_e = nc.declare_dram_parameter("cb", [L, CB_W], BF16, isOutput=False)
    Wp_e = nc.declare_dram_parameter("Wp", [RC, L, WS_W], BF16, isOutput=False)
    out_e = nc.declare_dram_parameter("out", [RC, FD], F32, isOutput=True)

    from contextlib import ExitStack

    with ExitStack() as ctx:
        e = ctx.enter_context
        cf = e(nc.sbuf_tensor([L, CF_W], F32))
        cb = e(nc.sbuf_tensor([L, CB_W], BF16))
        Xp = e(nc.sbuf_tensor([L, FD], BF16))
        dist = e(nc.sbuf_tensor([L, FD], F32))
        kbuf = e(nc.sbuf_tensor([L, KB * FD], BF16))
        wsbuf = e(nc.sbuf_tensor([L, WB * WS_W], BF16))
        g = e(nc.sbuf_tensor([L, FD], BF16))
        qb = e(nc.sbuf_tensor([L, PRB * FD], BF16))
        m0b = e(nc.sbuf_tensor([L, PRB * FD], BF16))
        m2b = e(nc.sbuf_tensor([L, PRB * FD], BF16))
        m3b = e(nc.sbuf_tensor([L, PRB * FD], BF16))
        latb = e(nc.sbuf_tensor([L, LAB * FD], BF16))
        wlbuf = e(nc.sbuf_tensor([L, LB * FD], BF16))
        ob = e(nc.sbuf_tensor([RC, FD], F32))
        outt = e(nc.sbuf_tensor([RC, FD], F32))
        psz = [e(nc.psum_tensor(f"psz{j}", [L, FD], F32)) for j in range(ZB)]
        ps0 = e(nc.psum_tensor([RC, 512], F32))
        ps1 = e(nc.psum_tensor([RC, 512], F32))
        cfsem = e(nc.semaphore("cfsem"))
        cbsem = e(nc.semaphore("cbsem"))
        wsem = [e(nc.semaphore(f"wsem{j}")) for j in range(WB)]
        wgsem = [e(nc.semaphore(f"wgsem{j}")) for j in range(3)]
        asem = e(nc.semaphore("asem"))   # ACT k completions (1/r)
        lsem = e(nc.semaphore("lsem"))   # ACT lat completions (1/r)
        zsem = e(nc.semaphore("zsem"))   # PE z-group completions (1/r)
        msem = e(nc.semaphore("msem"))   # PE out-mm completions (1/r)
        vsem = e(nc.semaphore("vsem"))   # DVE op completions
        osem = e(nc.semaphore("osem"))
        block = e(nc.Block())

        Tt = cf[:, 0:FD]
        Abc = cf[:, FD : FD + RC]
        NASbc = cf[:, FD + RC : FD + 2 * RC]
        LBVt = cf[0:RC, FD + 2 * RC : FD + 2 * RC + D]
        Xt = cb[:, 0:FD]
        Mt = cb[:, FD : 2 * FD]
        PDt = cb[:, 2 * FD : 3 * FD]
        OH3 = cb[:, 3 * FD : 3 * FD + RC * RC].rearrange("p (r m) -> p r m", r=RC)
        Ident = cb[:, 3 * FD + RC * RC :]

        def r3(ap):
            return ap.rearrange("p (b d) -> p b d", b=BC)

        def kslot(r):
            return kbuf[:, (r % KB) * FD : (r % KB + 1) * FD]

        def wslot(r):
            return wsbuf[:, (r % WB) * WS_W : (r % WB + 1) * WS_W]

        def wbc(r, e):
            base = (r % WB) * WS_W + e * D
            return wsbuf[:, base : base + D].unsqueeze(1).broadcast_to([L, BC, D])

        def bt4x(r):
            base = (r % WB) * WS_W + 5 * D
            return wsbuf[:, base : base + FD]

        def latslot(r):
            return latb[:, (r % LAB) * FD : (r % LAB + 1) * FD]

        def wlslot(r):
            return wlbuf[:, (r % LB) * FD : (r % LB + 1) * FD]

        def prod(buf, r):
            return buf[:, (r % PRB) * FD : (r % PRB + 1) * FD]

        CBH = CB_W // 2

        @block.gpsimd
        def _(gp):
            # first three ws slices on the otherwise-idle SWDGE ring so the
            # SP ring can stream cb immediately
            for r in range(3):
                gp.dma_start(out=wslot(r), in_=Wp_e[r, :, :]).then_inc(
                    wgsem[r], 16
                )

        @block.sync
        def _(sp):
            sp.dma_start(out=cb[:, 0:CBH], in_=cb_e[:, 0:CBH]).then_inc(cbsem, 16)
            sp.dma_start(out=cb[:, CBH:], in_=cb_e[:, CBH:]).then_inc(cbsem, 16)
            for r in range(3, RC):
                if r >= WB:
                    # ws slot readers: DVE wl(r-WB) is the last DVE read;
                    # PE z-group(r-WB) reads bt4x
                    sp.wait_ge(vsem, _vc_wl(r - WB))
                    sp.wait_ge(zsem, r - WB + 1)
                sp.dma_start(out=wslot(r), in_=Wp_e[r, :, :]).then_inc(
                    wsem[r % WB], 16
                )
            # split output: half 0 DMAs while half 1 is still computing
            sp.wait_ge(vsem, 75)
            sp.dma_start(out=out_e[:, 0:512], in_=outt[:, 0:512]).then_inc(osem, 16)
            sp.wait_ge(vsem, 77)
            sp.dma_start(out=out_e[:, 512:], in_=outt[:, 512:]).then_inc(osem, 16)

        @block.scalar
        def _(act):
            act.dma_start(out=cf[:, :], in_=cf_e[:, :]).then_inc(cfsem, 16)
            act.wait_ge(cfsem, 16)
            for r in range(RC):
                nc.scalar.activation(
                    dist[:, :], Tt, AF.Abs,
                    bias=NASbc[:, r : r + 1], scale=Abc[:, r : r + 1],
                )
                if r >= KB:
                    act.wait_ge(vsem, _vc_g(r - KB))
                nc.scalar.activation(
                    kslot(r), dist[:, :], AF.Exp, scale=-1.0
                ).then_inc(asem, 1)
                if r >= ALAG:
                    rr = r - ALAG
                    act.wait_ge(zsem, rr + 1)
                    if rr >= LAB:
                        act.wait_ge(vsem, _vc_wl(rr - LAB))
                    nc.scalar.activation(
                        latslot(rr), psz[rr % ZB][:, :], AF.Relu
                    ).then_inc(lsem, 1)
            for rr in range(RC - ALAG, RC):
                act.wait_ge(zsem, rr + 1)
                nc.scalar.activation(
                    latslot(rr), psz[rr % ZB][:, :], AF.Relu
                ).then_inc(lsem, 1)

        @block.vector
        def _(ve):
            ve.wait_ge(cbsem, 32)
            nc.vector.tensor_scalar_max(Xp[:, :], Xt, 0.0).then_inc(vsem, 1)
            for r in range(RC):
                if r >= WLAG:
                    rr = r - WLAG
                    # wl(rr): lat(rr) is WLAG iterations old; the lsem wait
                    # also implies zsem >= r-2, covering product-slot reuse
                    ve.wait_ge(lsem, rr + 1)
                    if rr >= LB:
                        ve.wait_ge(msem, rr - LB + 1)
                    nc.vector.tensor_tensor(
                        r3(wlslot(rr)), r3(latslot(rr)), wbc(rr, 4),
                        OP.mult,
                    ).then_inc(vsem, 1)
                ve.wait_ge(asem, r + 1)
                nc.vector.tensor_mul(g[:, :], Xp[:, :], kslot(r)).then_inc(vsem, 1)
                if r < 3:
                    ve.wait_ge(wgsem[r], 16)
                else:
                    ve.wait_ge(
                        wsem[r % WB],
                        16 * (r // WB + (0 if r % WB < 3 else 1)),
                    )
                nc.vector.tensor_tensor(
                    r3(prod(qb, r)), r3(g[:, :]), wbc(r, 1), OP.mult
                ).then_inc(vsem, 1)
                nc.vector.tensor_tensor(
                    r3(prod(m0b, r)), r3(Xt), wbc(r, 0), OP.mult
                ).then_inc(vsem, 1)
                nc.vector.tensor_tensor(
                    r3(prod(m2b, r)), r3(Mt), wbc(r, 2), OP.mult
                ).then_inc(vsem, 1)
                nc.vector.tensor_tensor(
                    r3(prod(m3b, r)), r3(PDt), wbc(r, 3), OP.mult
                ).then_inc(vsem, 1)
            # final wl's + tail
            for rr in range(RC - WLAG, RC):
                ve.wait_ge(lsem, rr + 1)
                ve.wait_ge(msem, rr - LB + 1)
                nc.vector.tensor_tensor(
                    r3(wlslot(rr)), r3(latslot(rr)), wbc(rr, 4), OP.mult
                ).then_inc(vsem, 1)
            ve.wait_ge(msem, RC)
            ve.wait_ge(cfsem, 16)
            lb3 = LBVt.unsqueeze(1).broadcast_to([RC, BC // 2, D])
            for h, ps in enumerate((ps0, ps1)):
                ob3 = r3(ob[:, :])[:, h * (BC // 2) : (h + 1) * (BC // 2), :]
                ps3 = ps[:, :].rearrange("p (b d) -> p b d", b=BC // 2)
                nc.vector.scalar_tensor_tensor(
                    ob3, ps3, 0.0, lb3, OP.add, OP.add
                ).then_inc(vsem, 1)
                nc.vector.tensor_scalar_max(
                    outt[:, h * 512 : (h + 1) * 512],
                    ob[:, h * 512 : (h + 1) * 512], 0.0,
                ).then_inc(vsem, 1)

        @block.tensor
        def _(te):
            te.wait_ge(cbsem, 32)
            # HAM warmup: ~12 throwaway matmuls on already-loaded SBUF so the
            # PE clock is at 2.4GHz when the first real z-group arrives; ps0
            # is clobbered but the real accumulation restarts with start=True
            for _w in range(12):
                nc.tensor.matmul(
                    ps0[:, :], OH3[:, 0, :], cb[:, 0:512],
                    start=True, stop=True, skip_group_check=True,
                )
            for r in range(RC):
                te.wait_ge(vsem, _vc_m3(r))
                if r >= ZB:
                    te.wait_ge(lsem, r - ZB + 1)
                pz = psz[r % ZB]
                # alternate PSUM banks between consecutive matmuls so the
                # drain of one overlaps the fill of the next
                for pb, first, last in (
                    (None, True, False),
                    (qb, False, False),
                    (m0b, False, False),
                    (m2b, False, False),
                    (m3b, False, True),
                ):
                    for h in range(2):
                        c0, c1 = h * 512, (h + 1) * 512
                        rhs = (
                            bt4x(r)[:, c0:c1]
                            if pb is None
                            else prod(pb, r)[:, c0:c1]
                        )
                        mm = nc.tensor.matmul(
                            pz[:, c0:c1], Ident, rhs,
                            start=first, stop=last, skip_group_check=True,
                        )
                        if last and h == 1:
                            mm.then_inc(zsem, 1)
                if r >= WLAG:
                    rr = r - WLAG
                    te.wait_ge(vsem, _vc_wl(rr))
                    wl = wlslot(rr)
                    nc.tensor.matmul(
                        ps0[:, :], OH3[:, rr, :], wl[:, 0:512],
                        start=(rr == 0), stop=False,
                        skip_group_check=True,
                    )
                    nc.tensor.matmul(
                        ps1[:, :], OH3[:, rr, :], wl[:, 512:1024],
                        start=(rr == 0), stop=False,
                        skip_group_check=True,
                    ).then_inc(msem, 1)
            for rr in range(RC - WLAG, RC):
                te.wait_ge(vsem, _vc_wl(rr))
                wl = wlslot(rr)
                nc.tensor.matmul(
                    ps0[:, :], OH3[:, rr, :], wl[:, 0:512],
                    start=False, stop=(rr == RC - 1), skip_group_check=True,
                )
                nc.tensor.matmul(
                    ps1[:, :], OH3[:, rr, :], wl[:, 512:1024],
                    start=False, stop=(rr == RC - 1), skip_group_check=True,
                ).then_inc(msem, 1)

    return nc


_CACHE = {}


def kernel(X, T, M, PD, alpha, w_v, w_t, b_t, b_v, ref_time):
    X = np.asarray(X, np.float32)
    T = np.asarray(T, np.float32)
    M = np.asarray(M, np.float32)
    PD = np.asarray(PD, np.float32)
    alpha = np.asarray(alpha, np.float32)
    w_v = np.asarray(w_v, np.float32)
    w_t = np.asarray(w_t, np.float32)
    b_t = np.asarray(b_t, np.float32)
    b_v = np.asarray(b_v, np.float32)
    ref_time = np.asarray(ref_time, np.float32)

    a = np.maximum(alpha.reshape(R), 0.0)
    s_ref = ref_time.reshape(R)
    nas = -(a * s_ref)
    bt4 = 4.0 * b_t[..., 0]              # [R, L, D]
    lbv = float(L) * b_v[:, 0, :]        # [R, D]

    # per-r params: [wt0|wt1|wt2|wt3|wv] (5*D) + bt4 expanded to [L, FD]
    wts = np.stack(
        [w_t[..., 0], w_t[..., 1], w_t[..., 2], w_t[..., 3], w_v], axis=2
    )                                     # [R, L, 5, D]
    bt4x = np.broadcast_to(bt4[:, :, None, :], (R, L, BC, D)).reshape(R, L, FD)
    wpack = np.concatenate(
        [wts.reshape(R, L, 5 * D), bt4x], axis=2
    )                                     # [R, L, WS_W]

    oh = np.zeros((L, RC, RC), np.float32)
    for r in range(RC):
        oh[:, r, r] = 1.0
    ident = np.eye(L, dtype=np.float32)

    if "nc" not in _CACHE:
        _CACHE["nc"] = _build_graph()
    nc = _CACHE["nc"]

    in_maps = []
    for c in range(8):
        b0 = (c // NR) * BC
        r0 = (c % NR) * RC
        tr = lambda x: np.ascontiguousarray(
            x[b0 : b0 + BC].transpose(1, 0, 2).reshape(L, FD)
        )
        cf = np.zeros((L, CF_W), np.float32)
        cf[:, 0:FD] = tr(T)
        cf[:, FD : FD + RC] = a[r0 : r0 + RC]
        cf[:, FD + RC : FD + 2 * RC] = nas[r0 : r0 + RC]
        cf[0:RC, FD + 2 * RC : FD + 2 * RC + D] = lbv[r0 : r0 + RC]
        cbf = np.zeros((L, CB_W), np.float32)
        cbf[:, 0:FD] = tr(X)
        cbf[:, FD : 2 * FD] = tr(M)
        cbf[:, 2 * FD : 3 * FD] = tr(PD)
        cbf[:, 3 * FD : 3 * FD + RC * RC] = oh.reshape(L, RC * RC)
        cbf[:, 3 * FD + RC * RC :] = ident
        in_maps.append(
            {
                "cf": cf,
                "cb": cbf.astype(_nbf16),
                "Wp": np.ascontiguousarray(wpack[r0 : r0 + RC]).astype(_nbf16),
            }
        )

    trace = bool(os.environ.get("BASS_KERNEL_TRACE"))
    kw = {}
    if trace:
        tmpdir = os.environ.get("BASS_KERNEL_TRACE_DIR") or None
        kw = dict(trace=True, tmpdir=tmpdir)
    res = run_bass_kernel_spmd(nc, in_maps, core_ids=list(range(8)), **kw)
    if trace:
        _CACHE["exec_time_ns"] = res.exec_time_ns
        print(f"HW exec time: {res.exec_time_ns} ns")

    out = np.zeros((B, R, D), np.float32)
    for c in range(8):
        b0 = (c // NR) * BC
        r0 = (c % NR) * RC
        o = np.asarray(res.results[c]["out"], np.float32).reshape(RC, BC, D)
        out[b0 : b0 + BC, r0 : r0 + RC] = o.transpose(1, 0, 2)
    return out
